# revision 12
# baseline (speedup 1.0000x reference)
"""LRU forward on 8 Trainium2 NeuronCores.

Sharding: 8 shards = 4 batches x 2 sequence halves (L_local = 2048).
Per-core dataflow is fully transposed (channels / d_model on SBUF partitions,
time on the free dim):

  input proj   Bu^T[j, m] = Bg_cat^T @ x^T          (float32r matmuls)
  scan         complex diagonal recurrence -> rotating frame e^{-i theta t}
               turns it into 4 real per-lane scans (hardware
               tensor_tensor_scan), carry between sequence halves exchanged
               with a pairwise AllReduce and applied as g += r^{s+1} * c_hat
  output proj  ys^T = CT_cat^T @ h2 (+ diag(D) blocks @ x^T for the skip path)

Host side only preprocesses/shards (transposes, small phase tables) and
reassembles the output.
"""

import os

import numpy as np

B, L, D, N = 4, 4096, 1024, 256
NCORE = 8
LLOC = L // 2          # per-core sequence length
MC = 512               # time chunk (matmul moving free dim)
NMC = LLOC // MC       # 4 chunks
N2 = 2 * N             # stacked re|im channels

_CACHE = {}
LAST_RESULTS = None    # test.py reads exec_time_ns from here


def _build():
    import concourse.bass as bass
    import concourse.mybir as mybir
    import concourse.tile as tile
    from concourse import bacc

    f32 = mybir.dt.float32
    f32r = mybir.dt.float32r
    ADD = mybir.AluOpType.add
    SUB = mybir.AluOpType.subtract
    MUL = mybir.AluOpType.mult

    use_f32r = os.environ.get("LRU_MM_F32", "0") != "1"
    mdt = f32r if use_f32r else f32

    nc = bacc.Bacc("TRN2", target_bir_lowering=False, debug=False, num_devices=NCORE)

    # ---- DRAM I/O (per-core) ----
    xT = nc.dram_tensor("xT", [D, LLOC], mdt, kind="ExternalInput").ap()
    bgd = nc.dram_tensor("bg", [D, N2], mdt, kind="ExternalInput").ap()
    ctd = nc.dram_tensor("ct", [N2, D], mdt, kind="ExternalInput").ap()
    costd = nc.dram_tensor("cost", [N, LLOC], f32, kind="ExternalInput").ap()
    sintd = nc.dram_tensor("sint", [N, LLOC], f32, kind="ExternalInput").ap()
    rbd = nc.dram_tensor("rb", [N, MC], f32, kind="ExternalInput").ap()
    rpowd = nc.dram_tensor("rpow", [N, MC], f32, kind="ExternalInput").ap()
    rfacd = nc.dram_tensor("rfac", [N, NMC], f32, kind="ExternalInput").ap()
    rot48d = nc.dram_tensor("rot48", [N, 2], f32, kind="ExternalInput").ap()
    gmd = nc.dram_tensor("gmask", [128, 4], f32, kind="ExternalInput").ap()
    pmd = nc.dram_tensor("pmask", [128, 4], f32, kind="ExternalInput").ap()
    ddd = nc.dram_tensor("ddiag", [D, 128], mdt, kind="ExternalInput").ap()
    outT = nc.dram_tensor("outT", [D, LLOC], f32, kind="ExternalOutput").ap()

    with tile.TileContext(nc) as tc:
        from contextlib import ExitStack

        with ExitStack() as st:
            cpool = st.enter_context(tc.tile_pool(name="consts", bufs=1))
            xpool = st.enter_context(tc.tile_pool(name="xt", bufs=1))
            gpool = st.enter_context(tc.tile_pool(name="g", bufs=1))
            spool = st.enter_context(tc.tile_pool(name="stream", bufs=int(os.environ.get("LRU_SBUFS","2"))))
            upool = st.enter_context(tc.tile_pool(name="u", bufs=2))
            hpool = st.enter_context(tc.tile_pool(name="h", bufs=1))
            opool = st.enter_context(tc.tile_pool(name="o", bufs=3))
            ps = st.enter_context(tc.tile_pool(name="ps", bufs=2, space="PSUM"))
            dram = st.enter_context(tc.tile_pool(name="dram", bufs=1, space="DRAM"))

            # ---- constant loads ----
            bg_sb = []
            for ki in range(8):
                t = cpool.tile([128, N2], mdt, tag=f"bg{ki}", name=f"bg{ki}")
                nc.sync.dma_start(t[:], bgd[128 * ki:128 * (ki + 1), :])
                bg_sb.append(t)
            ct_sb = []
            for tt_ in range(4):
                t = cpool.tile([128, D], mdt, tag=f"ct{tt_}", name=f"ct{tt_}")
                nc.sync.dma_start(t[:], ctd[128 * tt_:128 * (tt_ + 1), :])
                ct_sb.append(t)
            dd_sb = []
            for di in range(8):
                t = cpool.tile([128, 128], mdt, tag=f"dd{di}", name=f"dd{di}")
                nc.sync.dma_start(t[:], ddd[128 * di:128 * (di + 1), :])
                dd_sb.append(t)
            rb_sb, rpow_sb, rfac_sb, rot48_sb = [], [], [], []
            for c in range(2):
                rows = slice(128 * c, 128 * (c + 1))
                t = cpool.tile([128, MC], f32, tag=f"rb{c}", name=f"rb{c}")
                nc.sync.dma_start(t[:], rbd[rows, :])
                rb_sb.append(t)
                t = cpool.tile([128, MC], f32, tag=f"rp{c}", name=f"rp{c}")
                nc.sync.dma_start(t[:], rpowd[rows, :])
                rpow_sb.append(t)
                t = cpool.tile([128, NMC], f32, tag=f"rf{c}", name=f"rf{c}")
                nc.sync.dma_start(t[:], rfacd[rows, :])
                rfac_sb.append(t)
                t = cpool.tile([128, 2], f32, tag=f"r48{c}", name=f"r48{c}")
                nc.sync.dma_start(t[:], rot48d[rows, :])
                rot48_sb.append(t)
            gm_sb = cpool.tile([128, 4], f32, tag="gm", name="gm")
            nc.sync.dma_start(gm_sb[:], gmd[:, :])
            pm_sb = cpool.tile([128, 4], f32, tag="pm", name="pm")
            nc.sync.dma_start(pm_sb[:], pmd[:, :])

            xt_sb = []
            for ki in range(8):
                t = xpool.tile([128, LLOC], mdt, tag=f"xt{ki}", name=f"xt{ki}")
                nc.sync.dma_start(t[:], xT[128 * ki:128 * (ki + 1), :])
                xt_sb.append(t)

            g4 = []
            for tt_ in range(4):
                g4.append(gpool.tile([128, LLOC], f32, tag=f"g{tt_}", name=f"g{tt_}"))

            # ---- phase A: input projection + rot-in + scan, per time chunk ----
            for m in range(NMC):
                ms = slice(m * MC, (m + 1) * MC)
                pbu = []
                for j in range(4):
                    pt = ps.tile([128, MC], f32, tag=f"p{j}", name=f"bu{j}_{m}")
                    for ki in range(8):
                        nc.tensor.matmul(
                            pt[:],
                            bg_sb[ki][:, 128 * j:128 * (j + 1)],
                            xt_sb[ki][:, ms],
                            start=(ki == 0),
                            stop=(ki == 7),
                        )
                    pbu.append(pt)
                cs_sb, sn_sb = [], []
                for c in range(2):
                    rows = slice(128 * c, 128 * (c + 1))
                    t = spool.tile([128, MC], f32, tag=f"cs{c}", name=f"csA{c}_{m}")
                    nc.sync.dma_start(t[:], costd[rows, ms])
                    cs_sb.append(t)
                    t = spool.tile([128, MC], f32, tag=f"sn{c}", name=f"snA{c}_{m}")
                    nc.sync.dma_start(t[:], sintd[rows, ms])
                    sn_sb.append(t)
                if os.environ.get("LRU_ABL", "") == "norot":
                    for tt_ in range(4):
                        init = 0.0 if m == 0 else g4[tt_][:, m * MC - 1:m * MC]
                        nc.vector.tensor_tensor_scan(
                            g4[tt_][:, ms], rb_sb[tt_ & 1][:], pbu[tt_][:], init,
                            MUL, ADD)
                    continue
                u4 = [None] * 4
                for c in range(2):
                    u_re = upool.tile([128, MC], f32, tag=f"u{c}", name=f"u{c}_{m}")
                    nc.vector.tensor_tensor(u_re[:], pbu[c][:], cs_sb[c][:], MUL)
                    tmp = upool.tile([128, MC], f32, tag="tmp", name=f"tA{c}_{m}")
                    nc.vector.tensor_tensor(tmp[:], pbu[2 + c][:], sn_sb[c][:], MUL)
                    nc.vector.tensor_tensor(u_re[:], u_re[:], tmp[:], ADD)
                    u4[c] = u_re
                    u_im = upool.tile([128, MC], f32, tag=f"u{2+c}", name=f"u{2+c}_{m}")
                    nc.vector.tensor_tensor(u_im[:], pbu[2 + c][:], cs_sb[c][:], MUL)
                    tmp2 = upool.tile([128, MC], f32, tag="tmp", name=f"tB{c}_{m}")
                    nc.vector.tensor_tensor(tmp2[:], pbu[c][:], sn_sb[c][:], MUL)
                    nc.vector.tensor_tensor(u_im[:], u_im[:], tmp2[:], SUB)
                    u4[2 + c] = u_im
                for tt_ in range(4):
                    init = 0.0 if m == 0 else g4[tt_][:, m * MC - 1:m * MC]
                    nc.vector.tensor_tensor_scan(
                        g4[tt_][:, ms], rb_sb[tt_ & 1][:], u4[tt_][:], init, MUL, ADD
                    )

            # ---- phase B: carry exchange (pairwise AllReduce) ----
            stage = cpool.tile([128, 4], f32, tag="stage", name="stage")
            for tt_ in range(4):
                nc.vector.tensor_copy(stage[:, tt_:tt_ + 1], g4[tt_][:, LLOC - 1:LLOC])
            # scatter my carry into my pair's 4-column group (zero elsewhere)
            stage16 = cpool.tile([128, 16], f32, tag="stage16", name="stage16")
            for p in range(4):
                nc.vector.tensor_scalar_mul(
                    stage16[:, 4 * p:4 * (p + 1)], stage[:], gm_sb[:, p:p + 1])
            in_cc = dram.tile([128, 16], f32, tag="incc", name="incc")
            out_cc = dram.tile([128, 16], f32, tag="outcc", name="outcc",
                               addr_space="Shared")
            nc.gpsimd.dma_start(in_cc[:], stage16[:])
            if os.environ.get("LRU_NOCC", "0") == "1":
                # collective-free variant for TimelineSim bottleneck analysis
                nc.gpsimd.dma_start(out_cc[:], in_cc[:])
            else:
                nc.gpsimd.collective_compute(
                    "AllReduce",
                    mybir.AluOpType.add,
                    replica_groups=[list(range(NCORE))],
                    ins=[in_cc.opt()],
                    outs=[out_cc.opt()],
                )
            recv16 = cpool.tile([128, 16], f32, tag="recv16", name="recv16")
            nc.gpsimd.dma_start(recv16[:], out_cc[:])
            # select my pair's group: recv = sum_p recv16[:, 4p:4p+4] * pm[:, p]
            recv = cpool.tile([128, 4], f32, tag="recv", name="recv")
            nc.vector.tensor_scalar_mul(recv[:], recv16[:, 0:4], pm_sb[:, 0:1])
            for p in range(1, 4):
                nc.vector.scalar_tensor_tensor(
                    recv[:], recv16[:, 4 * p:4 * (p + 1)], pm_sb[:, p:p + 1],
                    recv[:], MUL, ADD)

            chat = cpool.tile([128, 4], f32, tag="chat", name="chat")
            tca = cpool.tile([128, 1], f32, tag="tca", name="tca")
            tcb = cpool.tile([128, 1], f32, tag="tcb", name="tcb")
            for c in range(2):
                c48 = rot48_sb[c][:, 0:1]
                s48 = rot48_sb[c][:, 1:2]
                # chat_re = recv_re*cos48 - recv_im*sin48
                nc.vector.tensor_tensor(tca[:], recv[:, c:c + 1], c48, MUL)
                nc.vector.tensor_tensor(tcb[:], recv[:, 2 + c:3 + c], s48, MUL)
                nc.vector.tensor_tensor(chat[:, c:c + 1], tca[:], tcb[:], SUB)
                # chat_im = recv_im*cos48 + recv_re*sin48
                nc.vector.tensor_tensor(tca[:], recv[:, 2 + c:3 + c], c48, MUL)
                nc.vector.tensor_tensor(tcb[:], recv[:, c:c + 1], s48, MUL)
                nc.vector.tensor_tensor(chat[:, 2 + c:3 + c], tca[:], tcb[:], ADD)
            chatm = cpool.tile([128, 16], f32, tag="chatm", name="chatm")
            for tt_ in range(4):
                for m in range(NMC):
                    nc.vector.tensor_tensor(
                        chatm[:, 4 * tt_ + m:4 * tt_ + m + 1],
                        chat[:, tt_:tt_ + 1],
                        rfac_sb[tt_ & 1][:, m:m + 1],
                        MUL,
                    )

            # ---- phase C: carry fix + rot-out + output projection ----
            for m in range(NMC):
                ms = slice(m * MC, (m + 1) * MC)
                for tt_ in range(4):
                    nc.vector.scalar_tensor_tensor(
                        g4[tt_][:, ms],
                        rpow_sb[tt_ & 1][:],
                        chatm[:, 4 * tt_ + m:4 * tt_ + m + 1],
                        g4[tt_][:, ms],
                        MUL,
                        ADD,
                    )
                cs_sb, sn_sb = [], []
                for c in range(2):
                    rows = slice(128 * c, 128 * (c + 1))
                    t = spool.tile([128, MC], f32, tag=f"cs{c}", name=f"csC{c}_{m}")
                    nc.sync.dma_start(t[:], costd[rows, ms])
                    cs_sb.append(t)
                    t = spool.tile([128, MC], f32, tag=f"sn{c}", name=f"snC{c}_{m}")
                    nc.sync.dma_start(t[:], sintd[rows, ms])
                    sn_sb.append(t)
                if os.environ.get("LRU_ABL", "") == "norot":
                    h4 = []
                    for tt_ in range(4):
                        hh = hpool.tile([128, MC], mdt, tag=f"h{tt_}", name=f"h{tt_}_{m}")
                        nc.scalar.copy(hh[:], g4[tt_][:, ms])
                        h4.append(hh)
                elif True:
                    h4 = [None] * 4
                else:
                    pass
                for c in (() if os.environ.get("LRU_ABL", "") == "norot" else range(2)):
                    h_re = hpool.tile([128, MC], mdt, tag=f"h{c}", name=f"h{c}_{m}")
                    nc.vector.tensor_tensor(h_re[:], g4[c][:, ms], cs_sb[c][:], MUL)
                    tmp = upool.tile([128, MC], f32, tag="tmp", name=f"tC{c}_{m}")
                    nc.vector.tensor_tensor(tmp[:], g4[2 + c][:, ms], sn_sb[c][:], MUL)
                    nc.vector.tensor_tensor(h_re[:], h_re[:], tmp[:], SUB)
                    h4[c] = h_re
                    h_im = hpool.tile([128, MC], mdt, tag=f"h{2+c}", name=f"h{2+c}_{m}")
                    nc.vector.tensor_tensor(h_im[:], g4[2 + c][:, ms], cs_sb[c][:], MUL)
                    tmp2 = upool.tile([128, MC], f32, tag="tmp", name=f"tD{c}_{m}")
                    nc.vector.tensor_tensor(tmp2[:], g4[c][:, ms], sn_sb[c][:], MUL)
                    nc.vector.tensor_tensor(h_im[:], h_im[:], tmp2[:], ADD)
                    h4[2 + c] = h_im
                for di in range(8):
                    pt = ps.tile([128, MC], f32, tag=f"p{di % 4}", name=f"o{di}_{m}")
                    for tt_ in range(4):
                        nc.tensor.matmul(
                            pt[:],
                            ct_sb[tt_][:, 128 * di:128 * (di + 1)],
                            h4[tt_][:],
                            start=(tt_ == 0),
                            stop=False,
                        )
                    nc.tensor.matmul(
                        pt[:], dd_sb[di][:], xt_sb[di][:, ms],
                        start=False, stop=True,
                    )
                    ot = opool.tile([128, MC], f32, tag="ot", name=f"ot{di}_{m}")
                    nc.scalar.copy(ot[:], pt[:])
                    nc.sync.dma_start(outT[128 * di:128 * (di + 1), ms], ot[:])

    nc.compile()
    return nc


def _prep(inputs):
    """Host-side parameter prep + sharding. Returns per-core input maps."""
    x = np.ascontiguousarray(np.asarray(inputs["input_sequence"], np.float32))
    nu_log = np.asarray(inputs["nu_log"], np.float32)
    theta_log = np.asarray(inputs["theta_log"], np.float32)
    B_re = np.asarray(inputs["B_re"], np.float32)
    B_im = np.asarray(inputs["B_im"], np.float32)
    C_re = np.asarray(inputs["C_re"], np.float32)
    C_im = np.asarray(inputs["C_im"], np.float32)
    Dv = np.asarray(inputs["D"], np.float32)

    r32 = np.exp(-np.exp(nu_log, dtype=np.float32), dtype=np.float32)
    th = np.exp(theta_log, dtype=np.float32).astype(np.float64)
    gamma = np.sqrt((1.0 - r32 * r32).astype(np.float32))

    bg = np.ascontiguousarray(
        np.concatenate([(gamma[:, None] * B_re).T, (gamma[:, None] * B_im).T], axis=1),
        np.float32)                                     # [D, 512]
    ct = np.ascontiguousarray(
        np.concatenate([C_re.T, -C_im.T], axis=0), np.float32)  # [512, D]

    t = np.arange(LLOC, dtype=np.float64)
    ang = th[:, None] * t[None, :]
    cost = np.cos(ang).astype(np.float32)
    sint = np.sin(ang).astype(np.float32)
    r64 = r32.astype(np.float64)
    s = np.arange(MC, dtype=np.float64)
    rpow = (r64[:, None] ** (s[None, :] + 1)).astype(np.float32)
    rfac = (r64[:, None] ** (MC * np.arange(NMC, dtype=np.float64)[None, :])).astype(
        np.float32)
    rb = np.ascontiguousarray(np.broadcast_to(r32[:, None], (N, MC)), np.float32)
    ph48 = th * float(LLOC)
    rot48 = np.stack([np.cos(ph48), np.sin(ph48)], axis=1).astype(np.float32)
    zrot = np.zeros_like(rot48)

    ddiag = np.zeros((D, 128), np.float32)
    for i in range(8):
        idx = np.arange(128)
        ddiag[128 * i + idx, idx] = Dv[128 * i + idx]

    in_maps = []
    for c in range(NCORE):
        b, h = c // 2, c % 2
        xTs = np.ascontiguousarray(x[b, h * LLOC:(h + 1) * LLOC, :].T)
        gm = np.zeros((128, 4), np.float32)
        pm = np.zeros((128, 4), np.float32)
        if h == 0:
            gm[:, b] = 1.0      # first-half core contributes to its pair's group
        pm[:, b] = 1.0          # every core selects its pair's group
        in_maps.append({
            "xT": xTs, "bg": bg, "ct": ct, "cost": cost, "sint": sint,
            "rb": rb, "rpow": rpow, "rfac": rfac,
            "rot48": (rot48 if h == 1 else zrot),
            "gmask": gm, "pmask": pm,
            "ddiag": ddiag,
        })
    return in_maps


def kernel(**inputs) -> np.ndarray:
    global LAST_RESULTS
    from concourse.bass_utils import run_bass_kernel_spmd

    if "nc" not in _CACHE:
        _CACHE["nc"] = _build()
    nc = _CACHE["nc"]

    in_maps = _prep(inputs)
    trace = os.environ.get("LRU_TRACE", "0") == "1"
    res = run_bass_kernel_spmd(
        nc, in_maps, core_ids=list(range(NCORE)), trace=trace,
        trace_cores=list(range(NCORE)) if trace else None,
        stitch_traces=trace,
    )
    LAST_RESULTS = res

    out = np.empty((B, L, D), np.float32)
    for c in range(NCORE):
        b, h = c // 2, c % 2
        out[b, h * LLOC:(h + 1) * LLOC, :] = res.results[c]["outT"].T
    return out
